# revision 17
# baseline (speedup 1.0000x reference)
"""Trainium2 Bass kernel for nn_CEDLTwoLoop100M (periodic-decay retention).

Strategy
--------
8 cores: core c owns batch b = c//4 and 3 head-slots.  Heads are grouped
by decay window so each SPMD slot compiles the tightest causal key
window shared by the heads bound to it across core groups:
  slot 0 <- heads {5,4,3,2}  (gamma<=0.931, window ~256)  KJMIN [0,2,6,10]
  slot 1 <- heads {1,0,1,0}  (gamma<=0.866, window ~129)  KJMIN [0,3,7,11]
  slot 2 <- heads {9,8,7,6}  (gamma up to 0.995, full)    KJMIN [0,0,0,0]
Key blocks whose decay factor is < 1e-8 are skipped entirely (81 of 120
blocks survive).  Duplicate head slots get zeroed w_out slices; the host
sums per-core partial outputs.

The decay*periodic kernel D[i,j] = g^(i-j) * cos(w(i-j)+phi) (causal) is
rank-2 per tile: folded into doubled Q'/K' features with per-(512,128)
block scales applied at PSUM evacuation (see v1/v2 notes).

Attention runs in ST-form (keys on partitions, queries free) so S@V
needs no transposes; rowsums of |S| come from a ones-matmul over an
ACT-produced |S| copy; normalization is applied to O.

v3 perf changes vs v2:
 - per-head decay-window truncation (above): 120 -> 81 key blocks.
 - output path in bf16: two partial streams y20 (slots 2+0, emitted
   during slot-1 attention) and y1 (tail), each [EC,128,T] bf16; host
   sums.  wout/h/gate tiles all bf16 (2x DVE modes, half DMA).
 - h-builds and GN chains interleaved into the following slot's
   attention loop (kills DVE head-of-line blocking that let the PE HAM
   re-throttle mid-kernel).
 - startup DMA issue spread across sync/scalar/gpsimd queues; weight
   and x streams issued before small constants.
"""

import math
import os
import numpy as np
import ml_dtypes

import concourse.bass as bass
import concourse.tile as tile
from concourse import bass_isa
from concourse import bacc, mybir
from concourse.bass_utils import run_bass_kernel_spmd

F32 = mybir.dt.float32
F32R = mybir.dt.float32r
BF16 = mybir.dt.bfloat16

B, T, D = 2, 2048, 640
K, DH = 10, 64
NCORES = 8
NSLOT = 3
EC = 5          # e (contraction) chunks of 128
TCH = 4         # token chunks of 512
NTB = 16        # token blocks of 128
GN_EPS = 1e-5

# head assignment per core-group (core % 4); same for both batches.
# column s (slot s) across groups shares one compiled key window.
HEADS = [[5, 1, 9], [4, 0, 8], [3, 1, 7], [2, 0, 6]]
ACTIVE = [[1, 1, 1], [1, 1, 1], [1, 0, 1], [1, 0, 1]]
# first key block per query chunk, per slot (decay < 1e-8 skipped)
KJMIN = {0: [0, 2, 6, 10], 1: [0, 3, 7, 11], 2: [0, 0, 0, 0]}

_PROGRAM_CACHE = {}
LAST_RESULTS = None


def _build_program():
    """Build the single SPMD Bass program (same for all 8 cores)."""
    nc = bacc.Bacc("TRN2", target_bir_lowering=False, debug=False)

    # ---- DRAM I/O ----------------------------------------------------
    xT_d = nc.dram_tensor("xT", [EC, 128, T], BF16, kind="ExternalInput")
    wfm_d = nc.dram_tensor("wfm", [EC, 128, 8, 128], BF16, kind="ExternalInput")
    wvg_d = nc.dram_tensor("wvg", [EC, 128, 192], BF16, kind="ExternalInput")
    wout20_d = nc.dram_tensor("wout20", [128, D], BF16, kind="ExternalInput")
    wout1_d = nc.dram_tensor("wout1", [64, D], BF16, kind="ExternalInput")
    qkrep_d = nc.dram_tensor("qkrep", [NSLOT, 2, 128, T], BF16, kind="ExternalInput")
    stab_d = nc.dram_tensor("stab", [128, NSLOT * 64], F32, kind="ExternalInput")
    pbias_d = nc.dram_tensor("pbias", [128, 8], F32, kind="ExternalInput")
    vbias_d = nc.dram_tensor("vbias", [128, 192], F32, kind="ExternalInput")
    gnw_d = nc.dram_tensor("gnw", [64, NSLOT], F32, kind="ExternalInput")
    gnb_d = nc.dram_tensor("gnb", [64, NSLOT], F32, kind="ExternalInput")
    triu_d = nc.dram_tensor("triu", [128, 128], F32, kind="ExternalInput")
    ones_d = nc.dram_tensor("ones", [128, 64], BF16, kind="ExternalInput")
    y20T_d = nc.dram_tensor("y20T", [EC, 128, T], BF16, kind="ExternalOutput")
    y1T_d = nc.dram_tensor("y1T", [EC, 128, T], BF16, kind="ExternalOutput")

    AL = mybir.AluOpType
    AF = mybir.ActivationFunctionType

    with tile.TileContext(nc) as tc, \
         nc.allow_low_precision(reason="bf16 matmul operands; accumulations in fp32 PSUM"):
        with (
            tc.tile_pool(name="consts", bufs=1) as consts,
            tc.tile_pool(name="persist", bufs=1) as persist,
            tc.tile_pool(name="ppsum", bufs=2, space="PSUM") as ppsum,
            tc.tile_pool(name="apsum", bufs=1, space="PSUM") as apsum,
            tc.tile_pool(name="work", bufs=2) as work,
            tc.tile_pool(name="rswork", bufs=2) as rswork,
        ):
            # ---- persistent intermediates ----------------------------
            qpt = [persist.tile([128, T], BF16, tag=f"qpt{s}", name=f"qpt{s}") for s in range(NSLOT)]
            kpt = [persist.tile([128, T], BF16, tag=f"kpt{s}", name=f"kpt{s}") for s in range(NSLOT)]
            vsb = persist.tile([128, NTB, 192], BF16, tag="vsb")
            gate01 = persist.tile([128, T], BF16, tag="gate01")
            gate2 = persist.tile([64, T], BF16, tag="gate2")
            h20 = persist.tile([128, T], BF16, tag="h20")
            h1 = persist.tile([64, T], BF16, tag="h1")
            osb = [persist.tile([64, T], F32, tag=f"osb{s}", name=f"osb{s}")
                   for s in range(NSLOT)]
            # per-slot alpha|beta (all at base partition 0)
            ab = [persist.tile([64, 2], F32, tag=f"ab{s}", name=f"ab{s}")
                  for s in range(NSLOT)]
            gn_s1 = [persist.tile([64, TCH], F32, tag=f"gns1_{s}", name=f"gns1_{s}") for s in range(NSLOT)]
            gn_s2 = [persist.tile([64, TCH], F32, tag=f"gns2_{s}", name=f"gns2_{s}") for s in range(NSLOT)]

            # ---- constants (issued after the first weight streams,
            #      earliest-needed first) -----------------------------
            C = {}

            def emit_consts():
                for tag, shape, dt, src in (
                    ("vbias", [128, 192], F32, vbias_d),
                    ("pbias", [128, 8], F32, pbias_d),
                    ("stab", [128, NSLOT * 64], F32, stab_d),
                    ("triu", [128, 128], F32, triu_d),
                    ("ones_t", [128, 64], BF16, ones_d),
                    ("gnw", [64, NSLOT], F32, gnw_d),
                    ("gnb", [64, NSLOT], F32, gnb_d),
                    ("wout20", [128, D], BF16, wout20_d),
                    ("wout1", [64, D], BF16, wout1_d),
                ):
                    C[tag] = consts.tile(shape, dt, tag=tag, name=tag)
                    nc.scalar.dma_start(C[tag][:], src[:])
                C["eps_t"] = consts.tile([64, 1], F32, tag="eps_t", name="eps_t")
                nc.gpsimd.memset(C["eps_t"][:], GN_EPS)

            SKEW = 3  # rowsum/SV matmuls trail the S matmul by this many blocks

            def attention_ic(s, ic):
                """One 512-query chunk of slot s's retention (skewed pipeline)."""
                stab, triu, ones_t = C['stab'], C['triu'], C['ones_t']
                ones_col = ones_t[:, 0:1]
                kjmin = KJMIN[s][ic]
                nkj = 4 * ic + 4
                ot = apsum.tile([128, 512], F32, tag="ot", bufs=3)
                rsp = ot[64:128, :]
                sts, asts, offs = {}, {}, {}

                def consume(kj):
                    off = offs[kj]
                    nc.tensor.matmul(
                        rsp[:, off:512], ones_t[:],
                        asts[kj][:, off:512],
                        start=(kj == kjmin), stop=(kj == nkj - 1),
                        skip_group_check=True,
                    )
                    nc.tensor.matmul(
                        ot[0:64, off:512],
                        vsb[:, kj, s * 64 : s * 64 + 64],
                        sts[kj][:, off:512],
                        start=(kj == kjmin), stop=(kj == nkj - 1),
                        skip_group_check=True,
                    )
                    del sts[kj], asts[kj]

                for kj in range(kjmin, nkj):
                    off = 128 * (kj - 4 * ic) if kj > 4 * ic else 0
                    n = 512 - off
                    stp = apsum.tile([128, 512], F32, tag="stp", bufs=3)
                    nc.tensor.matmul(
                        stp[:, off:512],
                        kpt[s][:, bass.ts(kj, 128)],
                        qpt[s][:, ic * 512 + off : (ic + 1) * 512],
                        start=True, stop=True,
                    )
                    st = work.tile([128, 512], BF16, tag="st", bufs=2 + SKEW)
                    ast = work.tile([128, 512], BF16, tag="ast", bufs=2 + SKEW)
                    sts[kj], asts[kj], offs[kj] = st, ast, off
                    sc_ap = stab[:, s * 64 + ic * 16 + kj : s * 64 + ic * 16 + kj + 1]
                    if kj >= 4 * ic:
                        # diagonal 128-block: mask with triu on DVE; ACT
                        # evacuates the causal remainder with the scale
                        nc.vector.scalar_tensor_tensor(
                            out=st[:, off : off + 128],
                            in0=stp[:, off : off + 128], scalar=sc_ap,
                            in1=triu[:], op0=AL.mult, op1=AL.mult,
                        )
                        if n > 128:
                            nc.scalar.activation(
                                st[:, off + 128 : 512], stp[:, off + 128 : 512],
                                AF.Identity, scale=sc_ap,
                            )
                    else:
                        # full block: ACT evacuates st (PSUM->SBUF w/ scale)
                        nc.scalar.activation(
                            st[:], stp[:],
                            AF.Identity, scale=sc_ap,
                        )
                    # |st| = max(-st, st) on DVE (bf16 TT-family, 2x mode)
                    nc.vector.scalar_tensor_tensor(
                        out=ast[:, off:512],
                        in0=st[:, off:512], scalar=-1.0,
                        in1=st[:, off:512], op0=AL.mult, op1=AL.max,
                    )
                    if kj - SKEW >= kjmin:
                        consume(kj - SKEW)
                for kj in range(max(kjmin, nkj - SKEW), nkj):
                    consume(kj)
                # rowsum arrives pre-broadcast on ot[64:128]: max(.,1) -> 1/x
                rmax = rswork.tile([64, 512], F32, tag="rmax")
                nc.vector.tensor_scalar(
                    out=rmax[:], in0=rsp[:, :], scalar1=1.0, scalar2=None,
                    op0=AL.max,
                )
                rinv = rswork.tile([64, 512], F32, tag="rinv")
                nc.vector.reciprocal_approx_fast(rinv[:], rmax[:])
                nc.vector.scalar_tensor_tensor(
                    out=osb[s][:, bass.ts(ic, 512)], in0=ot[0:64, :],
                    scalar=1.0, in1=rinv[:],
                    op0=AL.mult, op1=AL.mult,
                    accum_out=gn_s1[s][:, ic : ic + 1],
                )
                junk = work.tile([64, 512], F32, tag="junk")
                nc.scalar.activation(
                    junk[:], osb[s][:, bass.ts(ic, 512)],
                    AF.Square,
                    accum_out=gn_s2[s][:, ic : ic + 1],
                )

            def h_build(s, ic):
                """GN-apply + gate for one 512-chunk of slot s (bf16 out).

                gate01 layout: partitions 0:64 = slot 1's gate, 64:128 =
                slot 0's, so every tensor_tensor has matching input base
                partitions (NCC_IBIR297).
                """
                tsl = bass.ts(ic, 512)
                if s == 2:
                    gsrc, dst, lo = gate2[:, tsl], h20[0:64, tsl], True
                elif s == 0:
                    gsrc, dst, lo = gate01[64:128, tsl], h20[64:128, tsl], False
                else:
                    gsrc, dst, lo = gate01[0:64, tsl], h1[:, tsl], True
                tmpf = work.tile([128, 512], BF16, tag="htmp")
                tmp = tmpf[0:64, :] if lo else tmpf[64:128, :]
                nc.scalar.activation(
                    tmp, osb[s][:, tsl],
                    AF.Identity,
                    bias=ab[s][:, 1:2], scale=ab[s][:, 0:1],
                )
                nc.gpsimd.tensor_tensor(
                    out=dst, in0=tmp, in1=gsrc, op=AL.mult,
                )

            def y_mm(stream, tch):
                """Partial out-projection for one 512-chunk (bf16 to DRAM)."""
                wout20, wout1 = C['wout20'], C['wout1']
                for f in range(EC):
                    yp = ppsum.tile([128, 512], F32, tag="pps", name="yp")
                    if stream == 0:
                        nc.tensor.matmul(
                            yp[:], wout20[:, bass.ts(f, 128)],
                            h20[:, bass.ts(tch, 512)],
                            start=True, stop=True,
                        )
                        dst = y20T_d
                    else:
                        nc.tensor.matmul(
                            yp[:], wout1[:, bass.ts(f, 128)],
                            h1[:, bass.ts(tch, 512)],
                            start=True, stop=True,
                        )
                        dst = y1T_d
                    ysb = work.tile([128, 512], BF16, tag="ysb", bufs=4)
                    if f % 2 == 0:
                        nc.scalar.copy(ysb[:], yp[:])
                    else:
                        nc.vector.tensor_copy(ysb[:], yp[:])
                    nc.sync.dma_start(dst[f][:, bass.ts(tch, 512)], ysb[:])

            # ---- projections, fused with slot-2 attention per chunk ----
            with tc.tile_pool(name="projpool", bufs=1) as projpool, \
                 tc.tile_pool(name="xstream", bufs=2) as xstream, \
                 tc.tile_pool(name="reppool", bufs=1) as reppool:
                wfm = projpool.tile([128, EC, 8, 128], BF16, tag="wfm")
                wvg = projpool.tile([128, EC, 192], BF16, tag="wvg")
                for e in range(EC):
                    nc.gpsimd.dma_start(wvg[:, e], wvg_d[e])
                    nc.gpsimd.dma_start(wfm[:, e], wfm_d[e])
                for tch in range(TCH):
                    xts = xstream.tile([128, EC, 512], BF16, tag="xts")
                    for e in range(EC):
                        eng = nc.scalar if (tch == 0 and e >= 3) else nc.sync
                        eng.dma_start(xts[:, e], xT_d[e][:, bass.ts(tch, 512)])
                    if tch == 0:
                        emit_consts()
                    pbias, vbias = C['pbias'], C['vbias']

                    # V projection for the 4 token-blocks of this chunk
                    for tb4 in range(4):
                        ps = ppsum.tile([128, 512], F32, tag="pps")
                        for e in range(EC):
                            nc.tensor.matmul(
                                ps[:, :192],
                                xts[:, e, bass.ts(tb4, 128)],
                                wvg[:, e],
                                start=(e == 0), stop=(e == EC - 1),
                            )
                        nc.vector.scalar_tensor_tensor(
                            out=vsb[:, 4 * tch + tb4], in0=ps[:, :192], scalar=1.0,
                            in1=vbias[:], op0=AL.mult, op1=AL.add,
                        )

                    # feature-major projections (slot 2 first: its attention
                    # chunk is emitted at the end of this tch iteration)
                    for s in [2, 0, 1]:
                        for (cc, vr, dst) in ((s, 0, qpt[s]), (3 + s, 2, kpt[s])):
                            rep = reppool.tile([128, 512], BF16, tag="rep", bufs=2)
                            tsl = bass.ts(tch, 512)
                            nc.sync.dma_start(rep[:], qkrep_d[s, vr // 2][:, tsl])
                            ps = ppsum.tile([128, 512], F32, tag="pps")
                            for e in range(EC):
                                nc.tensor.matmul(
                                    ps[:], wfm[:, e, cc],
                                    xts[:, e],
                                    start=(e == 0), stop=(e == EC - 1),
                                )
                            if vr == 2:
                                # k-proj: ACT evacuates (+bias), GPSIMD
                                # multiplies in the decay/periodic vectors
                                ktmp = work.tile([128, 512], BF16, tag="ktmp",
                                                 bufs=2)
                                nc.scalar.activation(
                                    ktmp[:], ps[:], AF.Identity,
                                    bias=pbias[:, cc : cc + 1], scale=1.0,
                                )
                                nc.gpsimd.tensor_tensor(
                                    out=dst[:, tsl], in0=ktmp[:], in1=rep[:],
                                    op=AL.mult,
                                )
                            else:
                                nc.vector.scalar_tensor_tensor(
                                    out=dst[:, tsl], in0=ps[:],
                                    scalar=pbias[:, cc : cc + 1],
                                    in1=rep[:],
                                    op0=AL.add, op1=AL.mult,
                                )
                    for (cc, dst) in ((6, gate01[:]), (7, gate2[:])):
                        ps = ppsum.tile([128, 512], F32, tag="pps")
                        for e in range(EC):
                            nc.tensor.matmul(
                                ps[:], wfm[:, e, cc],
                                xts[:, e],
                                start=(e == 0), stop=(e == EC - 1),
                            )
                        pp = ps[:] if cc == 6 else ps[0:64]
                        dd = dst[:, bass.ts(tch, 512)]
                        bb = pbias[:, cc : cc + 1] if cc == 6 else pbias[0:64, cc : cc + 1]
                        nc.scalar.activation(
                            dd, pp, AF.Silu,
                            bias=bb, scale=1.0,
                        )
                    attention_ic(2, tch)

            def gn_finalize(s):
                """Per-slot GroupNorm stats -> alpha/beta (tiny ops)."""
                gnw, gnb, eps_t = C['gnw'], C['gnb'], C['eps_t']
                sums = rswork.tile([64, 2], F32, tag="sums")
                nc.vector.reduce_sum(sums[:, 0:1], gn_s1[s][:], axis=mybir.AxisListType.X)
                nc.vector.reduce_sum(sums[:, 1:2], gn_s2[s][:], axis=mybir.AxisListType.X)
                tot = rswork.tile([64, 2], F32, tag="tot")
                nc.gpsimd.partition_all_reduce(tot[:], sums[:], channels=64,
                                               reduce_op=bass_isa.ReduceOp.add)
                stats = rswork.tile([64, 2], F32, tag="stats")
                nc.vector.tensor_scalar(
                    out=stats[:], in0=tot[:], scalar1=1.0 / (DH * T),
                    scalar2=None, op0=AL.mult,
                )
                # var = E[o^2] - mu^2  (per-partition, all partitions equal)
                var = rswork.tile([64, 1], F32, tag="var")
                nc.vector.scalar_tensor_tensor(
                    out=var[:], in0=stats[:, 0:1], scalar=stats[:, 0:1],
                    in1=stats[:, 1:2], op0=AL.mult, op1=AL.subtract,
                )
                nc.vector.tensor_scalar(
                    out=var[:], in0=var[:], scalar1=-1.0, scalar2=None, op0=AL.mult,
                )
                std = rswork.tile([64, 1], F32, tag="std")
                nc.scalar.activation(
                    std[:], var[:], AF.Sqrt,
                    bias=eps_t[:], scale=1.0,
                )
                rstd = rswork.tile([64, 1], F32, tag="rstd")
                nc.vector.reciprocal_approx_fast(rstd[:], std[:])
                alpha = rswork.tile([64, 1], F32, tag="alpha")
                nc.vector.tensor_tensor(
                    out=alpha[:], in0=gnw[:, s : s + 1], in1=rstd[:], op=AL.mult,
                )
                beta = rswork.tile([64, 1], F32, tag="beta")
                nc.vector.scalar_tensor_tensor(
                    out=beta[:], in0=stats[:, 0:1], scalar=alpha[:, 0:1],
                    in1=gnb[:, s : s + 1], op0=AL.mult, op1=AL.subtract,
                )
                nc.vector.tensor_scalar(
                    out=beta[:], in0=beta[:], scalar1=-1.0, scalar2=None,
                    op0=AL.mult,
                )
                nc.vector.tensor_copy(ab[s][:, 0:1], alpha[:])
                nc.vector.tensor_copy(ab[s][:, 1:2], beta[:])

            gn_finalize(2)
            # slot 0 attention; slot 2's gated GN chunks built in between
            # (keeps the DVE queue mixed so the PE never starves)
            for ic in range(TCH):
                attention_ic(0, ic)
                h_build(2, ic)
            gn_finalize(0)
            # slot 1 attention with slot-0 h chunks + the y20 partial
            # out-projection interleaved (PE stays dense through slot 1's
            # shorter window)
            for ic in range(TCH):
                attention_ic(1, ic)
                h_build(0, ic)
                y_mm(0, ic)
            gn_finalize(1)

            # ---- tail: slot-1 GN apply + out-proj ---------------------
            for tch in range(TCH):
                h_build(1, tch)
                y_mm(1, tch)

    nc.all_engine_barrier()
    nc.finalize()
    return nc


def _host_vectors(gamma_log, log_lambda, phi, heads):
    """Per-slot qc/qs/kc/ks vectors + block scale table (float64 math)."""
    i = np.arange(T, dtype=np.float64)
    vecs = np.zeros((12, T), np.float64)
    stab = np.zeros((NSLOT, TCH, 16), np.float64)
    for s, h in enumerate(heads):
        g = 1.0 / (1.0 + math.exp(-float(gamma_log[h])))
        lg = math.log(g)
        w = 2.0 * math.pi / math.exp(float(log_lambda[h]))
        ph = float(phi[h])
        vecs[4 * s + 0] = np.exp(lg * (i % 512)) * np.cos(w * i + ph)
        vecs[4 * s + 1] = np.exp(lg * (i % 512)) * np.sin(w * i + ph)
        vecs[4 * s + 2] = np.exp(-lg * (i % 128)) * np.cos(w * i)
        vecs[4 * s + 3] = np.exp(-lg * (i % 128)) * np.sin(w * i)
        for ic in range(TCH):
            for kj in range(4 * ic + 4):
                stab[s, ic, kj] = math.exp(lg * (512 * ic - 128 * kj))
    return vecs, stab.reshape(NSLOT * 64).astype(np.float32)


def _host_inputs(core, inp):
    """Build the per-core input map."""
    cb = core // 4
    grp = core % 4
    heads = HEADS[grp]
    active = ACTIVE[grp]

    bf = ml_dtypes.bfloat16
    x = np.asarray(inp["x"], np.float32)
    m = {}
    m["xT"] = np.ascontiguousarray(x[cb].T).reshape(EC, 128, T).astype(bf)

    def rows(wname, h):
        return np.asarray(inp[wname], np.float32)[64 * h : 64 * h + 64, :]

    chunks = []
    for s in range(NSLOT):
        q = rows("w_q_w", heads[s])
        chunks.append(np.concatenate([q, q], 0))
    for s in range(NSLOT):
        k = rows("w_k_w", heads[s])
        chunks.append(np.concatenate([k, k], 0))
    chunks.append(np.concatenate([rows("gate_w", heads[1]),
                                  rows("gate_w", heads[0])], 0))
    chunks.append(np.concatenate([rows("gate_w", heads[2]),
                                  np.zeros((64, D), np.float32)], 0))
    wall = np.concatenate(chunks, 0)          # (1024, 640) rows=out chans
    m["wfm"] = np.ascontiguousarray(wall.T).reshape(EC, 128, 8, 128).astype(bf)

    wv = np.concatenate([rows("w_v_w", heads[s]) for s in range(NSLOT)], 0)  # (192, 640)
    m["wvg"] = np.ascontiguousarray(wv.T).reshape(EC, 128, 192).astype(bf)

    wo = np.asarray(inp["w_out_w"], np.float32)
    wo_s = [np.ascontiguousarray(wo[:, 64 * heads[s] : 64 * heads[s] + 64].T)
            * np.float32(active[s]) for s in range(NSLOT)]
    m["wout20"] = np.concatenate([wo_s[2], wo_s[0]], 0).astype(bf)
    m["wout1"] = wo_s[1].astype(bf)

    vecs, stab = _host_vectors(np.asarray(inp["gamma_log"]),
                               np.asarray(inp["log_lambda"]),
                               np.asarray(inp["phi"]), heads)
    qkrep = np.zeros((NSLOT, 2, 128, T), np.float64)
    for s in range(NSLOT):
        qkrep[s, 0, 0:64, :] = vecs[4 * s + 0][None, :]
        qkrep[s, 0, 64:128, :] = vecs[4 * s + 1][None, :]
        qkrep[s, 1, 0:64, :] = vecs[4 * s + 2][None, :]
        qkrep[s, 1, 64:128, :] = vecs[4 * s + 3][None, :]
    m["qkrep"] = qkrep.astype(bf)
    m["stab"] = np.broadcast_to(stab, (128, NSLOT * 64)).copy()

    def bvec(name, h):
        return np.asarray(inp[name], np.float32)[64 * h : 64 * h + 64]

    pb = np.zeros((128, 8), np.float32)
    for s in range(NSLOT):
        pb[0:64, s] = bvec("w_q_b", heads[s])
        pb[64:128, s] = bvec("w_q_b", heads[s])
        pb[0:64, 3 + s] = bvec("w_k_b", heads[s])
        pb[64:128, 3 + s] = bvec("w_k_b", heads[s])
    pb[0:64, 6] = bvec("gate_b", heads[1])
    pb[64:128, 6] = bvec("gate_b", heads[0])
    pb[0:64, 7] = bvec("gate_b", heads[2])
    m["pbias"] = pb

    vb = np.zeros((192,), np.float32)
    for s in range(NSLOT):
        vb[64 * s : 64 * s + 64] = bvec("w_v_b", heads[s])
    m["vbias"] = np.broadcast_to(vb, (128, 192)).copy()

    gnw = np.stack([bvec("gn_weight", heads[s]) for s in range(NSLOT)], 1)
    gnb = np.stack([bvec("gn_bias", heads[s]) for s in range(NSLOT)], 1)
    m["gnw"] = np.ascontiguousarray(gnw)
    m["gnb"] = np.ascontiguousarray(gnb)
    m["triu"] = np.triu(np.ones((128, 128), np.float32))
    m["ones"] = np.ones((128, 64), bf)
    return m


def kernel(**inputs):
    global LAST_RESULTS
    key = "prog"
    if key not in _PROGRAM_CACHE:
        _PROGRAM_CACHE[key] = _build_program()
    nc = _PROGRAM_CACHE[key]

    in_maps = [_host_inputs(c, inputs) for c in range(NCORES)]
    res = run_bass_kernel_spmd(
        nc, in_maps, core_ids=list(range(NCORES)),
        trace=bool(os.environ.get("BASS_TRACE")),
    )
    LAST_RESULTS = res

    y = np.zeros((B, T, D), np.float32)
    for c in range(NCORES):
        cb = c // 4
        yT = res.results[c]["y20T"].astype(np.float32).reshape(D, T)
        y1T = res.results[c]["y1T"].astype(np.float32).reshape(D, T)
        y[cb] += yT.T
        y[cb] += y1T.T
    y += np.asarray(inputs["w_out_b"], np.float32)[None, None, :]
    return y


# revision 19
# speedup vs baseline: 1.2631x; 1.2631x over previous
"""Trainium2 Bass kernel for nn_CEDLTwoLoop100M (periodic-decay retention).

Strategy
--------
8 cores: core c owns batch b = c//4 and 3 head-slots.  Heads are grouped
by decay window so each SPMD slot compiles the tightest causal key
window shared by the heads bound to it across core groups:
  slot 0 <- heads {5,4,3,2}  (gamma<=0.931, window ~256)  KJMIN [0,2,6,10]
  slot 1 <- heads {9,8,7,6}  (gamma up to 0.995, full)    KJMIN [0,0,0,0]
  slot 2 <- heads {1,0,1,0}  (gamma<=0.866, window ~129)  KJMIN [0,3,7,11]
Key blocks whose decay factor is < 1e-8 are skipped entirely (81 of 120
blocks survive).  Duplicate head slots get zeroed w_out slices; the host
sums per-core partial outputs.

The decay*periodic kernel D[i,j] = g^(i-j) * cos(w(i-j)+phi) (causal) is
rank-2 per tile: folded into doubled Q'/K' features with per-(512,128)
block scales applied at PSUM evacuation (see v1/v2 notes).

Attention runs in ST-form (keys on partitions, queries free) so S@V
needs no transposes; rowsums of |S| come from a ones-matmul over an
ACT-produced |S| copy; normalization is applied to O.

v3 perf changes vs v2:
 - per-head decay-window truncation (above): 120 -> 81 key blocks.
 - output path in bf16: two partial streams y20 (slots 2+0, emitted
   during slot-1 attention) and y1 (tail), each [EC,128,T] bf16; host
   sums.  wout/h/gate tiles all bf16 (2x DVE modes, half DMA).
 - h-builds and GN chains interleaved into the following slot's
   attention loop (kills DVE head-of-line blocking that let the PE HAM
   re-throttle mid-kernel).
 - startup DMA issue spread across sync/scalar/gpsimd queues; weight
   and x streams issued before small constants.
"""

import math
import os
import numpy as np
import ml_dtypes

import concourse.bass as bass
import concourse.tile as tile
from concourse import bass_isa
from concourse import bacc, mybir
from concourse.bass_utils import run_bass_kernel_spmd

F32 = mybir.dt.float32
F32R = mybir.dt.float32r
BF16 = mybir.dt.bfloat16

B, T, D = 2, 2048, 640
K, DH = 10, 64
NCORES = 8
NSLOT = 3
EC = 5          # e (contraction) chunks of 128
TCH = 4         # token chunks of 512
NTB = 16        # token blocks of 128
GN_EPS = 1e-5

# head assignment per core-group (core % 4); same for both batches.
# column s (slot s) across groups shares one compiled key window.
HEADS = [[5, 9, 1], [4, 8, 0], [3, 7, 1], [2, 6, 0]]
ACTIVE = [[1, 1, 1], [1, 1, 1], [1, 1, 0], [1, 1, 0]]
# first key block per query chunk, per slot (decay < 1e-8 skipped).
# slot 2 (smallest window, DVE-heavy diag blocks) overlaps the PE-heavy
# projection phase; slot 1 (full window, ACT-friendly full blocks) runs
# last where the out-projection stream keeps the PE warm.
KJMIN = {0: [0, 2, 6, 10], 1: [0, 0, 0, 0], 2: [0, 3, 7, 11]}

_PROGRAM_CACHE = {}
LAST_RESULTS = None


def _build_program():
    """Build the single SPMD Bass program (same for all 8 cores)."""
    nc = bacc.Bacc("TRN2", target_bir_lowering=False, debug=False)

    # ---- DRAM I/O ----------------------------------------------------
    xT_d = nc.dram_tensor("xT", [EC, 128, T], BF16, kind="ExternalInput")
    wfm_d = nc.dram_tensor("wfm", [EC, 128, 8, 128], BF16, kind="ExternalInput")
    wvg_d = nc.dram_tensor("wvg", [EC, 128, 192], BF16, kind="ExternalInput")
    wout20_d = nc.dram_tensor("wout20", [128, D], BF16, kind="ExternalInput")
    wout1_d = nc.dram_tensor("wout1", [64, D], BF16, kind="ExternalInput")
    qkrep_d = nc.dram_tensor("qkrep", [NSLOT, 2, 128, T], BF16, kind="ExternalInput")
    stab_d = nc.dram_tensor("stab", [128, NSLOT * 64], F32, kind="ExternalInput")
    pbias_d = nc.dram_tensor("pbias", [128, 8], F32, kind="ExternalInput")
    vbias_d = nc.dram_tensor("vbias", [128, 192], F32, kind="ExternalInput")
    gnw_d = nc.dram_tensor("gnw", [64, NSLOT], F32, kind="ExternalInput")
    gnb_d = nc.dram_tensor("gnb", [64, NSLOT], F32, kind="ExternalInput")
    triu_d = nc.dram_tensor("triu", [128, 128], F32, kind="ExternalInput")
    ones_d = nc.dram_tensor("ones", [128, 64], BF16, kind="ExternalInput")
    y20T_d = nc.dram_tensor("y20T", [EC, 128, T], BF16, kind="ExternalOutput")
    y1T_d = nc.dram_tensor("y1T", [EC, 128, T], BF16, kind="ExternalOutput")

    AL = mybir.AluOpType
    AF = mybir.ActivationFunctionType

    with tile.TileContext(nc) as tc, \
         nc.allow_low_precision(reason="bf16 matmul operands; accumulations in fp32 PSUM"):
        with (
            tc.tile_pool(name="consts", bufs=1) as consts,
            tc.tile_pool(name="persist", bufs=1) as persist,
            tc.tile_pool(name="ppsum", bufs=2, space="PSUM") as ppsum,
            tc.tile_pool(name="apsum", bufs=1, space="PSUM") as apsum,
            tc.tile_pool(name="work", bufs=2) as work,
            tc.tile_pool(name="rswork", bufs=2) as rswork,
        ):
            # ---- persistent intermediates ----------------------------
            qpt = [persist.tile([128, T], BF16, tag=f"qpt{s}", name=f"qpt{s}") for s in range(NSLOT)]
            kpt = [persist.tile([128, T], BF16, tag=f"kpt{s}", name=f"kpt{s}") for s in range(NSLOT)]
            vsb = persist.tile([128, NTB, 192], BF16, tag="vsb")
            gate01 = persist.tile([128, T], BF16, tag="gate01")
            gate2 = persist.tile([64, T], BF16, tag="gate2")
            h20 = persist.tile([128, T], BF16, tag="h20")
            h1 = persist.tile([64, T], BF16, tag="h1")
            osb = [persist.tile([64, T], F32, tag=f"osb{s}", name=f"osb{s}")
                   for s in range(NSLOT)]
            # per-slot alpha|beta (all at base partition 0)
            ab = [persist.tile([64, 2], F32, tag=f"ab{s}", name=f"ab{s}")
                  for s in range(NSLOT)]
            gn_s1 = [persist.tile([64, TCH], F32, tag=f"gns1_{s}", name=f"gns1_{s}") for s in range(NSLOT)]
            gn_s2 = [persist.tile([64, TCH], F32, tag=f"gns2_{s}", name=f"gns2_{s}") for s in range(NSLOT)]

            # ---- constants (issued after the first weight streams,
            #      earliest-needed first) -----------------------------
            C = {}

            def emit_consts():
                for tag, shape, dt, src in (
                    ("vbias", [128, 192], F32, vbias_d),
                    ("pbias", [128, 8], F32, pbias_d),
                    ("stab", [128, NSLOT * 64], F32, stab_d),
                    ("triu", [128, 128], F32, triu_d),
                    ("ones_t", [128, 64], BF16, ones_d),
                    ("gnw", [64, NSLOT], F32, gnw_d),
                    ("gnb", [64, NSLOT], F32, gnb_d),
                    ("wout20", [128, D], BF16, wout20_d),
                    ("wout1", [64, D], BF16, wout1_d),
                ):
                    C[tag] = consts.tile(shape, dt, tag=tag, name=tag)
                    nc.scalar.dma_start(C[tag][:], src[:])
                C["eps_t"] = consts.tile([64, 1], F32, tag="eps_t", name="eps_t")
                nc.gpsimd.memset(C["eps_t"][:], GN_EPS)

            SKEW = 3  # rowsum/SV matmuls trail the S matmul by this many blocks

            def attention_ic(s, ic):
                """One 512-query chunk of slot s's retention (skewed pipeline)."""
                stab, triu, ones_t = C['stab'], C['triu'], C['ones_t']
                ones_col = ones_t[:, 0:1]
                kjmin = KJMIN[s][ic]
                nkj = 4 * ic + 4
                ot = apsum.tile([128, 512], F32, tag="ot", bufs=3)
                rsp = ot[64:128, :]
                sts, asts, offs = {}, {}, {}

                def consume(kj):
                    off = offs[kj]
                    nc.tensor.matmul(
                        rsp[:, off:512], ones_t[:],
                        asts[kj][:, off:512],
                        start=(kj == kjmin), stop=(kj == nkj - 1),
                        skip_group_check=True,
                    )
                    nc.tensor.matmul(
                        ot[0:64, off:512],
                        vsb[:, kj, s * 64 : s * 64 + 64],
                        sts[kj][:, off:512],
                        start=(kj == kjmin), stop=(kj == nkj - 1),
                        skip_group_check=True,
                    )
                    del sts[kj], asts[kj]

                for kj in range(kjmin, nkj):
                    off = 128 * (kj - 4 * ic) if kj > 4 * ic else 0
                    n = 512 - off
                    stp = apsum.tile([128, 512], F32, tag="stp", bufs=3)
                    nc.tensor.matmul(
                        stp[:, off:512],
                        kpt[s][:, bass.ts(kj, 128)],
                        qpt[s][:, ic * 512 + off : (ic + 1) * 512],
                        start=True, stop=True,
                    )
                    st = work.tile([128, 512], BF16, tag="st", bufs=2 + SKEW)
                    ast = work.tile([128, 512], BF16, tag="ast", bufs=2 + SKEW)
                    sts[kj], asts[kj], offs[kj] = st, ast, off
                    sc_ap = stab[:, s * 64 + ic * 16 + kj : s * 64 + ic * 16 + kj + 1]
                    if kj >= 4 * ic:
                        # diagonal 128-block: mask with triu on DVE; ACT
                        # evacuates the causal remainder with the scale
                        nc.vector.scalar_tensor_tensor(
                            out=st[:, off : off + 128],
                            in0=stp[:, off : off + 128], scalar=sc_ap,
                            in1=triu[:], op0=AL.mult, op1=AL.mult,
                        )
                        if n > 128:
                            nc.scalar.activation(
                                st[:, off + 128 : 512], stp[:, off + 128 : 512],
                                AF.Identity, scale=sc_ap,
                            )
                    else:
                        # full block: ACT evacuates st (PSUM->SBUF w/ scale)
                        nc.scalar.activation(
                            st[:], stp[:],
                            AF.Identity, scale=sc_ap,
                        )
                    # |st| = max(-st, st) on DVE (bf16 TT-family, 2x mode)
                    nc.vector.scalar_tensor_tensor(
                        out=ast[:, off:512],
                        in0=st[:, off:512], scalar=-1.0,
                        in1=st[:, off:512], op0=AL.mult, op1=AL.max,
                    )
                    if kj - SKEW >= kjmin:
                        consume(kj - SKEW)
                for kj in range(max(kjmin, nkj - SKEW), nkj):
                    consume(kj)
                # rowsum arrives pre-broadcast on ot[64:128]: max(.,1) -> 1/x
                rmax = rswork.tile([64, 512], F32, tag="rmax")
                nc.vector.tensor_scalar(
                    out=rmax[:], in0=rsp[:, :], scalar1=1.0, scalar2=None,
                    op0=AL.max,
                )
                rinv = rswork.tile([64, 512], F32, tag="rinv")
                nc.vector.reciprocal_approx_fast(rinv[:], rmax[:])
                nc.vector.scalar_tensor_tensor(
                    out=osb[s][:, bass.ts(ic, 512)], in0=ot[0:64, :],
                    scalar=1.0, in1=rinv[:],
                    op0=AL.mult, op1=AL.mult,
                    accum_out=gn_s1[s][:, ic : ic + 1],
                )
                junk = work.tile([64, 512], F32, tag="junk")
                nc.scalar.activation(
                    junk[:], osb[s][:, bass.ts(ic, 512)],
                    AF.Square,
                    accum_out=gn_s2[s][:, ic : ic + 1],
                )

            def h_build(s, ic):
                """GN-apply + gate for one 512-chunk of slot s (bf16 out).

                gate01 layout: partitions 0:64 = slot 1's gate, 64:128 =
                slot 0's, so every tensor_tensor has matching input base
                partitions (NCC_IBIR297).
                """
                tsl = bass.ts(ic, 512)
                if s == 2:
                    gsrc, dst, lo = gate2[:, tsl], h20[0:64, tsl], True
                elif s == 0:
                    gsrc, dst, lo = gate01[64:128, tsl], h20[64:128, tsl], False
                else:
                    gsrc, dst, lo = gate01[0:64, tsl], h1[:, tsl], True
                tmpf = work.tile([128, 512], BF16, tag="htmp")
                tmp = tmpf[0:64, :] if lo else tmpf[64:128, :]
                nc.scalar.activation(
                    tmp, osb[s][:, tsl],
                    AF.Identity,
                    bias=ab[s][:, 1:2], scale=ab[s][:, 0:1],
                )
                nc.vector.tensor_tensor(
                    out=dst, in0=tmp, in1=gsrc, op=AL.mult,
                )

            def y_mm(stream, tch):
                """Partial out-projection for one 512-chunk (bf16 to DRAM)."""
                wout20, wout1 = C['wout20'], C['wout1']
                for f in range(EC):
                    yp = ppsum.tile([128, 512], F32, tag="pps", name="yp")
                    if stream == 0:
                        nc.tensor.matmul(
                            yp[:], wout20[:, bass.ts(f, 128)],
                            h20[:, bass.ts(tch, 512)],
                            start=True, stop=True,
                        )
                        dst = y20T_d
                    else:
                        nc.tensor.matmul(
                            yp[:], wout1[:, bass.ts(f, 128)],
                            h1[:, bass.ts(tch, 512)],
                            start=True, stop=True,
                        )
                        dst = y1T_d
                    ysb = work.tile([128, 512], BF16, tag="ysb", bufs=4)
                    if f % 2 == 0:
                        nc.scalar.copy(ysb[:], yp[:])
                    else:
                        nc.vector.tensor_copy(ysb[:], yp[:])
                    nc.sync.dma_start(dst[f][:, bass.ts(tch, 512)], ysb[:])

            # ---- projections, fused with slot-2 attention per chunk ----
            with tc.tile_pool(name="projpool", bufs=1) as projpool, \
                 tc.tile_pool(name="xstream", bufs=2) as xstream, \
                 tc.tile_pool(name="reppool", bufs=1) as reppool:
                wfm = projpool.tile([128, EC, 8, 128], BF16, tag="wfm")
                wvg = projpool.tile([128, EC, 192], BF16, tag="wvg")
                for e in range(EC):
                    nc.gpsimd.dma_start(wvg[:, e], wvg_d[e])
                    nc.gpsimd.dma_start(wfm[:, e], wfm_d[e])
                for tch in range(TCH):
                    xts = xstream.tile([128, EC, 512], BF16, tag="xts")
                    for e in range(EC):
                        eng = nc.scalar if (tch == 0 and e >= 3) else nc.sync
                        eng.dma_start(xts[:, e], xT_d[e][:, bass.ts(tch, 512)])
                    if tch == 0:
                        emit_consts()
                    pbias, vbias = C['pbias'], C['vbias']

                    # V projection for the 4 token-blocks of this chunk
                    for tb4 in range(4):
                        ps = ppsum.tile([128, 512], F32, tag="pps")
                        for e in range(EC):
                            nc.tensor.matmul(
                                ps[:, :192],
                                xts[:, e, bass.ts(tb4, 128)],
                                wvg[:, e],
                                start=(e == 0), stop=(e == EC - 1),
                            )
                        nc.vector.scalar_tensor_tensor(
                            out=vsb[:, 4 * tch + tb4], in0=ps[:, :192], scalar=1.0,
                            in1=vbias[:], op0=AL.mult, op1=AL.add,
                        )

                    # feature-major projections (slot 2 first: its attention
                    # chunk is emitted at the end of this tch iteration)
                    for s in [2, 0, 1]:
                        for (cc, vr, dst) in ((s, 0, qpt[s]), (3 + s, 2, kpt[s])):
                            rep = reppool.tile([128, 512], BF16, tag="rep", bufs=2)
                            tsl = bass.ts(tch, 512)
                            nc.sync.dma_start(rep[:], qkrep_d[s, vr // 2][:, tsl])
                            ps = ppsum.tile([128, 512], F32, tag="pps")
                            for e in range(EC):
                                nc.tensor.matmul(
                                    ps[:], wfm[:, e, cc],
                                    xts[:, e],
                                    start=(e == 0), stop=(e == EC - 1),
                                )
                            nc.vector.scalar_tensor_tensor(
                                out=dst[:, tsl], in0=ps[:],
                                scalar=pbias[:, cc : cc + 1],
                                in1=rep[:],
                                op0=AL.add, op1=AL.mult,
                            )
                    for (cc, dst) in ((6, gate01[:]), (7, gate2[:])):
                        ps = ppsum.tile([128, 512], F32, tag="pps")
                        for e in range(EC):
                            nc.tensor.matmul(
                                ps[:], wfm[:, e, cc],
                                xts[:, e],
                                start=(e == 0), stop=(e == EC - 1),
                            )
                        pp = ps[:] if cc == 6 else ps[0:64]
                        dd = dst[:, bass.ts(tch, 512)]
                        bb = pbias[:, cc : cc + 1] if cc == 6 else pbias[0:64, cc : cc + 1]
                        nc.scalar.activation(
                            dd, pp, AF.Silu,
                            bias=bb, scale=1.0,
                        )
                    attention_ic(2, tch)

            def gn_finalize(s):
                """Per-slot GroupNorm stats -> alpha/beta (tiny ops)."""
                gnw, gnb, eps_t = C['gnw'], C['gnb'], C['eps_t']
                sums = rswork.tile([64, 2], F32, tag="sums")
                nc.vector.reduce_sum(sums[:, 0:1], gn_s1[s][:], axis=mybir.AxisListType.X)
                nc.vector.reduce_sum(sums[:, 1:2], gn_s2[s][:], axis=mybir.AxisListType.X)
                tot = rswork.tile([64, 2], F32, tag="tot")
                nc.gpsimd.partition_all_reduce(tot[:], sums[:], channels=64,
                                               reduce_op=bass_isa.ReduceOp.add)
                stats = rswork.tile([64, 2], F32, tag="stats")
                nc.vector.tensor_scalar(
                    out=stats[:], in0=tot[:], scalar1=1.0 / (DH * T),
                    scalar2=None, op0=AL.mult,
                )
                # var = E[o^2] - mu^2  (per-partition, all partitions equal)
                var = rswork.tile([64, 1], F32, tag="var")
                nc.vector.scalar_tensor_tensor(
                    out=var[:], in0=stats[:, 0:1], scalar=stats[:, 0:1],
                    in1=stats[:, 1:2], op0=AL.mult, op1=AL.subtract,
                )
                nc.vector.tensor_scalar(
                    out=var[:], in0=var[:], scalar1=-1.0, scalar2=None, op0=AL.mult,
                )
                std = rswork.tile([64, 1], F32, tag="std")
                nc.scalar.activation(
                    std[:], var[:], AF.Sqrt,
                    bias=eps_t[:], scale=1.0,
                )
                rstd = rswork.tile([64, 1], F32, tag="rstd")
                nc.vector.reciprocal_approx_fast(rstd[:], std[:])
                alpha = rswork.tile([64, 1], F32, tag="alpha")
                nc.vector.tensor_tensor(
                    out=alpha[:], in0=gnw[:, s : s + 1], in1=rstd[:], op=AL.mult,
                )
                beta = rswork.tile([64, 1], F32, tag="beta")
                nc.vector.scalar_tensor_tensor(
                    out=beta[:], in0=stats[:, 0:1], scalar=alpha[:, 0:1],
                    in1=gnb[:, s : s + 1], op0=AL.mult, op1=AL.subtract,
                )
                nc.vector.tensor_scalar(
                    out=beta[:], in0=beta[:], scalar1=-1.0, scalar2=None,
                    op0=AL.mult,
                )
                nc.vector.tensor_copy(ab[s][:, 0:1], alpha[:])
                nc.vector.tensor_copy(ab[s][:, 1:2], beta[:])

            gn_finalize(2)
            # slot 0 attention; slot 2's gated GN chunks built in between
            # (keeps the DVE queue mixed so the PE never starves)
            for ic in range(TCH):
                attention_ic(0, ic)
                h_build(2, ic)
            gn_finalize(0)
            # slot 1 attention with slot-0 h chunks + the y20 partial
            # out-projection interleaved (PE stays dense through slot 1's
            # shorter window)
            for ic in range(TCH):
                attention_ic(1, ic)
                h_build(0, ic)
                if ic < 3:
                    y_mm(0, ic)
            gn_finalize(1)

            # ---- tail: deferred y20 chunk covers the GN chain, then
            #      slot-1 GN apply + out-proj ------------------------
            y_mm(0, 3)
            for tch in range(TCH):
                h_build(1, tch)
                y_mm(1, tch)

    nc.all_engine_barrier()
    nc.finalize()
    return nc


def _host_vectors(gamma_log, log_lambda, phi, heads):
    """Per-slot qc/qs/kc/ks vectors + block scale table (float64 math)."""
    i = np.arange(T, dtype=np.float64)
    vecs = np.zeros((12, T), np.float64)
    stab = np.zeros((NSLOT, TCH, 16), np.float64)
    for s, h in enumerate(heads):
        g = 1.0 / (1.0 + math.exp(-float(gamma_log[h])))
        lg = math.log(g)
        w = 2.0 * math.pi / math.exp(float(log_lambda[h]))
        ph = float(phi[h])
        vecs[4 * s + 0] = np.exp(lg * (i % 512)) * np.cos(w * i + ph)
        vecs[4 * s + 1] = np.exp(lg * (i % 512)) * np.sin(w * i + ph)
        vecs[4 * s + 2] = np.exp(-lg * (i % 128)) * np.cos(w * i)
        vecs[4 * s + 3] = np.exp(-lg * (i % 128)) * np.sin(w * i)
        for ic in range(TCH):
            for kj in range(4 * ic + 4):
                stab[s, ic, kj] = math.exp(lg * (512 * ic - 128 * kj))
    return vecs, stab.reshape(NSLOT * 64).astype(np.float32)


def _host_inputs(core, inp):
    """Build the per-core input map."""
    cb = core // 4
    grp = core % 4
    heads = HEADS[grp]
    active = ACTIVE[grp]

    bf = ml_dtypes.bfloat16
    x = np.asarray(inp["x"], np.float32)
    m = {}
    m["xT"] = np.ascontiguousarray(x[cb].T).reshape(EC, 128, T).astype(bf)

    def rows(wname, h):
        return np.asarray(inp[wname], np.float32)[64 * h : 64 * h + 64, :]

    chunks = []
    for s in range(NSLOT):
        q = rows("w_q_w", heads[s])
        chunks.append(np.concatenate([q, q], 0))
    for s in range(NSLOT):
        k = rows("w_k_w", heads[s])
        chunks.append(np.concatenate([k, k], 0))
    chunks.append(np.concatenate([rows("gate_w", heads[1]),
                                  rows("gate_w", heads[0])], 0))
    chunks.append(np.concatenate([rows("gate_w", heads[2]),
                                  np.zeros((64, D), np.float32)], 0))
    wall = np.concatenate(chunks, 0)          # (1024, 640) rows=out chans
    m["wfm"] = np.ascontiguousarray(wall.T).reshape(EC, 128, 8, 128).astype(bf)

    wv = np.concatenate([rows("w_v_w", heads[s]) for s in range(NSLOT)], 0)  # (192, 640)
    m["wvg"] = np.ascontiguousarray(wv.T).reshape(EC, 128, 192).astype(bf)

    wo = np.asarray(inp["w_out_w"], np.float32)
    wo_s = [np.ascontiguousarray(wo[:, 64 * heads[s] : 64 * heads[s] + 64].T)
            * np.float32(active[s]) for s in range(NSLOT)]
    m["wout20"] = np.concatenate([wo_s[2], wo_s[0]], 0).astype(bf)
    m["wout1"] = wo_s[1].astype(bf)

    vecs, stab = _host_vectors(np.asarray(inp["gamma_log"]),
                               np.asarray(inp["log_lambda"]),
                               np.asarray(inp["phi"]), heads)
    qkrep = np.zeros((NSLOT, 2, 128, T), np.float64)
    for s in range(NSLOT):
        qkrep[s, 0, 0:64, :] = vecs[4 * s + 0][None, :]
        qkrep[s, 0, 64:128, :] = vecs[4 * s + 1][None, :]
        qkrep[s, 1, 0:64, :] = vecs[4 * s + 2][None, :]
        qkrep[s, 1, 64:128, :] = vecs[4 * s + 3][None, :]
    m["qkrep"] = qkrep.astype(bf)
    m["stab"] = np.broadcast_to(stab, (128, NSLOT * 64)).copy()

    def bvec(name, h):
        return np.asarray(inp[name], np.float32)[64 * h : 64 * h + 64]

    pb = np.zeros((128, 8), np.float32)
    for s in range(NSLOT):
        pb[0:64, s] = bvec("w_q_b", heads[s])
        pb[64:128, s] = bvec("w_q_b", heads[s])
        pb[0:64, 3 + s] = bvec("w_k_b", heads[s])
        pb[64:128, 3 + s] = bvec("w_k_b", heads[s])
    pb[0:64, 6] = bvec("gate_b", heads[1])
    pb[64:128, 6] = bvec("gate_b", heads[0])
    pb[0:64, 7] = bvec("gate_b", heads[2])
    m["pbias"] = pb

    vb = np.zeros((192,), np.float32)
    for s in range(NSLOT):
        vb[64 * s : 64 * s + 64] = bvec("w_v_b", heads[s])
    m["vbias"] = np.broadcast_to(vb, (128, 192)).copy()

    gnw = np.stack([bvec("gn_weight", heads[s]) for s in range(NSLOT)], 1)
    gnb = np.stack([bvec("gn_bias", heads[s]) for s in range(NSLOT)], 1)
    m["gnw"] = np.ascontiguousarray(gnw)
    m["gnb"] = np.ascontiguousarray(gnb)
    m["triu"] = np.triu(np.ones((128, 128), np.float32))
    m["ones"] = np.ones((128, 64), bf)
    return m


def kernel(**inputs):
    global LAST_RESULTS
    key = "prog"
    if key not in _PROGRAM_CACHE:
        _PROGRAM_CACHE[key] = _build_program()
    nc = _PROGRAM_CACHE[key]

    in_maps = [_host_inputs(c, inputs) for c in range(NCORES)]
    res = run_bass_kernel_spmd(
        nc, in_maps, core_ids=list(range(NCORES)),
        trace=bool(os.environ.get("BASS_TRACE")),
    )
    LAST_RESULTS = res

    y = np.zeros((B, T, D), np.float32)
    for c in range(NCORES):
        cb = c // 4
        yT = res.results[c]["y20T"].astype(np.float32).reshape(D, T)
        y1T = res.results[c]["y1T"].astype(np.float32).reshape(D, T)
        y[cb] += yT.T
        y[cb] += y1T.T
    y += np.asarray(inputs["w_out_b"], np.float32)[None, None, :]
    return y
